# revision 1
# baseline (speedup 1.0000x reference)
"""Trainium2 Bass kernel for an EoMT transformer encoder layer.

Layer (per batch element):
    xn  = LN1(x);  qkv = xn @ qkv_w;  masked softmax attention (16 heads);
    y   = attn_out @ proj_w + proj_b;  x1 = x + y
    h   = gelu(LN2(x1) @ fc1_w + fc1_b);  y2 = h @ fc2_w + fc2_b; out = x1 + y2

Sharding: pure data-parallel over batch — B=8 maps 1:1 onto the 8 NeuronCores,
no collectives.  Each core runs the full layer for its batch element.

Per-core layout choices:
  - qkv computed in two parts: q,k in TRANSPOSED layout qk^T [2D, S]
    (stationary = qkv_w chunk, moving = xn^T) so per-head q^T,k^T [64, S]
    feed the scores matmul directly; v in NATURAL layout [S, D] (stationary =
    xn^T chunk, moving = w_v) so it is the attn@v stationary directly.
  - scores are computed transposed, scoresT [k_pos, q_pos] = k^T.T @ q^T, so
    the softmax sum over k_pos falls out of a matmul against a ones column
    appended to v (denominator for free), flash-style per k-tile:
    scores -> exp (ACT reads PSUM, writes bf16 SBUF) -> mask-mul -> attn@v.
    No max-subtraction (|score*scale| < ~3 by construction).
  - out^T [D, S] = v.T @ expT accumulates per head; psum row 64 is the
    denominator.  Normalization multiplies by a DRAM-broadcast reciprocal.
  - fc1 emits h^T [MLP, S] with gelu+bias fused into the PSUM->SBUF
    activation; fc2 contracts h^T with fc2_w back to natural [S, D].
All matmuls are bf16 (weights pre-cast on host, activations cast on chip),
accumulating fp32 in PSUM.  LN statistics are fp32.
"""

import os
import sys

for _p in ("/opt/trn_rl_repo", "/root/.axon_site/_ro/trn_rl_repo"):
    if _p not in sys.path and os.path.isdir(_p):
        sys.path.append(_p)

import numpy as np
import ml_dtypes

import concourse.bass as bass
import concourse.tile as tile
from concourse import bacc
from concourse import mybir
from concourse.masks import make_identity

AFT = mybir.ActivationFunctionType
ALU = mybir.AluOpType
BF16 = mybir.dt.bfloat16
F32 = mybir.dt.float32

P = 128


class Cfg:
    def __init__(self, B=8, S=1124, D=1024, NP=1024, NQ=100, MLP=4096,
                 EPS=1e-6, use_ln1_g=False, use_ln1_b=False, use_ln2_g=False,
                 use_ln2_b=False, use_proj_b=False, use_fc2_b=False,
                 gelu=True):
        self.B, self.S, self.D = B, S, D
        self.NP, self.NQ, self.MLP, self.EPS = NP, NQ, MLP, EPS
        self.DH = 64
        self.H = D // self.DH
        assert D % P == 0 and MLP % P == 0
        self.SCALE = self.DH ** -0.5
        self.use_ln1_g, self.use_ln1_b = use_ln1_g, use_ln1_b
        self.use_ln2_g, self.use_ln2_b = use_ln2_g, use_ln2_b
        self.use_proj_b, self.use_fc2_b = use_proj_b, use_fc2_b
        self.gelu = gelu

    def key(self):
        return tuple(sorted((k, v) for k, v in self.__dict__.items()))


def _s_tiles(S):
    return [(i * P, min(P, S - i * P)) for i in range((S + P - 1) // P)]


def _chunks(N, width=512):
    return [(i * width, min(width, N - i * width))
            for i in range((N + width - 1) // width)]


def build_layer(nc, cfg, io):
    """Trace the layer program into `nc`.  `io` maps names to DRAM APs."""
    S, D, H, MLP, NP, NQ = cfg.S, cfg.D, cfg.H, cfg.MLP, cfg.NP, cfg.NQ
    ND = D // P                      # contraction chunks of D
    NQK = 2 * D // P                 # m-tiles of transposed q|k
    NM = MLP // P                    # m-tiles of MLP hidden
    stiles = _s_tiles(S)
    NS = len(stiles)
    qch = _chunks(S)                 # free chunks of S, <=512, bank-aligned
    dch = _chunks(D)                 # free chunks of D
    psw = 512 * max(len(qch), len(dch))

    WTW = 1024 if (3 * D) % 1024 == 0 else 3 * D   # qkv weight tile width
    n_qkvw = ND * (3 * D // WTW)
    WBUFS = max(n_qkvw, NM) + 2

    x_d, out_d, maskT_d = io["x"], io["out"], io["maskT"]

    with tile.TileContext(nc) as tc:
        with (
            tc.tile_pool(name="const", bufs=1) as cpool,
            tc.tile_pool(name="rp", bufs=1) as rp,
            tc.tile_pool(name="wp", bufs=1) as wp,
            tc.tile_pool(name="st", bufs=1) as st,
            tc.tile_pool(name="dp", bufs=1, space="DRAM") as dp,
            tc.tile_pool(name="ps", bufs=1, space="PSUM") as ps,
        ):
            RT = dict(tag="r", bufs=41)
            WT = dict(tag="w", bufs=WBUFS)

            x1_d = [dp.tile([rows, D], F32, name=f"x1_scr{i}", tag=f"x1{i}")
                    for i, (s0, rows) in enumerate(stiles)]
            den_d = [dp.tile([1, S], BF16, name=f"den_scr{h}", tag=f"den{h}")
                     for h in range(H)]

            ident = cpool.tile([P, P], BF16, name="ident")
            make_identity(nc, ident[:])
            eps_t = cpool.tile([P, 1], F32, name="eps")
            nc.vector.memset(eps_t, cfg.EPS)

            def bcast_vec(name, ap_1d):
                t = cpool.tile([P, ap_1d.shape[0]], F32, name=name)
                src = bass.AP(tensor=ap_1d.tensor, offset=ap_1d.offset,
                              ap=[[0, P]] + list(ap_1d.ap))
                nc.sync.dma_start(out=t[:], in_=src)
                return t

            ln1_g = bcast_vec("ln1_g", io["ln1_g"]) if cfg.use_ln1_g else None
            ln1_b = bcast_vec("ln1_b", io["ln1_b"]) if cfg.use_ln1_b else None
            ln2_g = bcast_vec("ln2_g", io["ln2_g"]) if cfg.use_ln2_g else None
            ln2_b = bcast_vec("ln2_b", io["ln2_b"]) if cfg.use_ln2_b else None
            proj_b = bcast_vec("proj_b", io["proj_b"]) if cfg.use_proj_b else None
            fc2_b = bcast_vec("fc2_b", io["fc2_b"]) if cfg.use_fc2_b else None

            fc1_b_sb = cpool.tile([P, NM], F32, name="fc1_b_sb")
            nc.sync.dma_start(out=fc1_b_sb[:],
                              in_=io["fc1_b"].rearrange("(mo ki) -> ki mo", ki=P))

            # binarized transposed mask per (partially) masked k-tile
            mtiles = []
            for kt, (k0, krows) in enumerate(stiles):
                if k0 >= NP or NQ == 0:
                    mtiles.append(None)
                    continue
                mrows = min(k0 + krows, NP) - k0
                mf = st.tile([P, NQ], F32, name=f"mf{kt}", tag="mf", bufs=1)
                nc.gpsimd.dma_start(out=mf[:mrows], in_=maskT_d[k0:k0 + mrows])
                mb = st.tile([P, NQ], BF16, name=f"mb{kt}", tag="mb", bufs=NS)
                nc.vector.tensor_scalar(out=mb[:mrows], in0=mf[:mrows],
                                        scalar1=0.5, scalar2=None,
                                        op0=ALU.is_gt)
                mtiles.append(mb)

            def psum_mm():
                psum_mm.i ^= 1
                return ps.tile([P, psw], F32, name=f"pmm{psum_mm.i}",
                               tag=f"mm{psum_mm.i}", bufs=1)
            psum_mm.i = 0

            # ---------------- LN + transpose helpers ----------------
            def layer_norm(x_t, srows, g, b, name):
                nsub = 2 if D > 512 else 1
                half = D // nsub
                stats = st.tile([P, nsub, 6], F32, name=f"sta{name}",
                                tag="stats", bufs=3)
                mv = st.tile([P, 2], F32, name=f"mv{name}", tag="mv", bufs=3)
                for i in range(nsub):
                    nc.vector.bn_stats(out=stats[:srows, i],
                                       in_=x_t[:srows, i * half:(i + 1) * half])
                nc.vector.bn_aggr(out=mv[:srows], in_=stats[:srows])
                std = st.tile([P, 1], F32, name=f"std{name}", tag="std", bufs=3)
                nc.scalar.activation(out=std[:srows], in_=mv[:srows, 1:2],
                                     func=AFT.Sqrt, bias=eps_t[:srows],
                                     scale=1.0)
                nc.vector.reciprocal(out=std[:srows], in_=std[:srows])
                xn_t = st.tile([P, D], BF16, name=f"xn{name}", tag="xn", bufs=2)
                nc.vector.tensor_scalar(out=xn_t[:srows], in0=x_t[:srows],
                                        scalar1=mv[:srows, 0:1],
                                        scalar2=std[:srows],
                                        op0=ALU.subtract, op1=ALU.mult)
                if g is not None:
                    nc.vector.tensor_mul(out=xn_t[:srows], in0=xn_t[:srows],
                                         in1=g[:srows])
                if b is not None:
                    nc.vector.tensor_add(out=xn_t[:srows], in0=xn_t[:srows],
                                         in1=b[:srows])
                return xn_t

            def transpose_into(xn_t, srows, s0, dst_tiles):
                for j in range(ND):
                    pt = ps.tile([P, P], BF16, name=f"ptr{j}", tag="tr", bufs=2)
                    nc.tensor.transpose(pt[:, :srows],
                                        xn_t[:srows, j * P:(j + 1) * P],
                                        ident[:srows, :srows])
                    nc.vector.tensor_copy(out=dst_tiles[j][:, s0:s0 + srows],
                                          in_=pt[:, :srows])

            # ---------------- LN1 ----------------
            xnT = [rp.tile([P, S], BF16, name=f"xnT{j}", **RT)
                   for j in range(ND)]
            for (s0, srows) in stiles:
                x_t = st.tile([P, D], F32, name="x_t", tag="xf", bufs=4)
                nc.gpsimd.dma_start(out=x_t[:srows], in_=x_d[s0:s0 + srows])
                xn_t = layer_norm(x_t, srows, ln1_g, ln1_b, f"1_{s0}")
                transpose_into(xn_t, srows, s0, xnT)

            # ---------------- qkv + attention (interleaved) ----------------
            # qkv/v matmuls accumulate in small 1-bank psums on the "tr" ring
            # so the PE can fill attention's exp-latency gaps with qkv work
            # while scores ("mm0") and attn_o ("mm1") hold the big psums.
            qkvw = {}
            qkv_w3 = io["qkv_w"].rearrange("(ko ki) n -> ki ko n", ki=P)
            for ko in range(ND):
                for wi in range(3 * D // WTW):
                    t = wp.tile([P, WTW], BF16, name=f"qkvw{ko}_{wi}", **WT)
                    nc.sync.dma_start(
                        out=t[:], in_=qkv_w3[:, ko, wi * WTW:(wi + 1) * WTW])
                    qkvw[(ko, wi)] = t

            def w_col(ko, m0, width):
                wi, off = divmod(m0, WTW)
                assert off + width <= WTW
                return qkvw[(ko, wi)][:, off:off + width]

            def small_mm(lhsT_fn, rhs_fn, out_rows, chunks, copy_fn):
                """Accumulate over ko into a 1-bank psum per free chunk."""
                for (c0, cn) in chunks:
                    pm = ps.tile([P, 512], F32, name="ptr_mm", tag="tr", bufs=2)
                    for ko in range(ND):
                        nc.tensor.matmul(pm[:out_rows, :cn],
                                         lhsT=lhsT_fn(ko),
                                         rhs=rhs_fn(ko, c0, cn),
                                         start=(ko == 0), stop=(ko == ND - 1))
                    copy_fn(pm, c0, cn)

            # v natural [S, D] with interleaved ones column (attn@v stationary)
            v_sb = []
            for kt in range(NS):
                t = rp.tile([P, H, 65], BF16, name=f"v{kt}", **RT)
                nc.vector.memset(t[:, :, 64:65], 1.0)
                v_sb.append(t)
            for kt, (s0, srows) in enumerate(stiles):
                def vcopy(pm, c0, cn, kt=kt, srows=srows):
                    h0 = c0 // 64
                    nc.vector.tensor_copy(
                        out=v_sb[kt][:srows, h0:h0 + cn // 64, 0:64],
                        in_=pm[:srows, :cn].rearrange("p (h d) -> p h d", d=64))
                small_mm(lambda ko, s0=s0, srows=srows: xnT[ko][:, s0:s0 + srows],
                         lambda ko, c0, cn: w_col(ko, 2 * D + c0, cn),
                         srows, dch, vcopy)

            qkT = [None] * NQK

            def emit_qk_tile(mt):
                t = rp.tile([P, S], BF16, name=f"qkT{mt}", **RT)

                def qkcopy(pm, c0, cn):
                    nc.vector.tensor_copy(out=t[:, c0:c0 + cn],
                                          in_=pm[:, :cn])
                small_mm(lambda ko, mt=mt: w_col(ko, mt * P, P),
                         lambda ko, c0, cn: xnT[ko][:, c0:c0 + cn],
                         P, qch, qkcopy)
                qkT[mt] = t

            outT = [rp.tile([P, S], BF16, name=f"outT{j}", **RT)
                    for j in range(ND)]

            def emit_head(h):
                qbase, kbase = h * 64, D + h * 64
                qT = qkT[qbase // P][qbase % P:qbase % P + 64, :]
                kT = qkT[kbase // P][kbase % P:kbase % P + 64, :]
                po = ps.tile([P, psw], F32, name=f"po{h}", tag="mm1", bufs=1)
                for kt, (k0, krows) in enumerate(stiles):
                    pm = ps.tile([P, psw], F32, name=f"pss{h}_{kt}", tag="mm0",
                                 bufs=1)
                    for ci, (q0, qn) in enumerate(qch):
                        nc.tensor.matmul(pm[:krows, ci * 512:ci * 512 + qn],
                                         lhsT=kT[:, k0:k0 + krows],
                                         rhs=qT[:, q0:q0 + qn],
                                         start=True, stop=True)
                    e = rp.tile([P, S], BF16, name=f"e{h}_{kt}", tag="expT",
                                bufs=4)
                    nc.scalar.activation(out=e[:krows, :S], in_=pm[:krows, :S],
                                         func=AFT.Exp, scale=cfg.SCALE)
                    if mtiles[kt] is not None:
                        mrows = min(k0 + krows, NP) - k0
                        nc.vector.tensor_mul(out=e[:mrows, NP:S],
                                             in0=e[:mrows, NP:S],
                                             in1=mtiles[kt][:mrows])
                    for ci, (q0, qn) in enumerate(qch):
                        nc.tensor.matmul(po[:65, ci * 512:ci * 512 + qn],
                                         lhsT=v_sb[kt][:krows, h, :],
                                         rhs=e[:krows, q0:q0 + qn],
                                         start=(kt == 0), stop=(kt == NS - 1))
                off = (h % 2) * 64
                od = outT[h // 2][off:off + 64, :]
                dt_ = st.tile([P, S], BF16, name=f"dt{h}", tag="dt", bufs=1)
                nc.vector.tensor_copy(out=dt_[64:65, :S], in_=po[64:65, :S])
                if off == 0:
                    nc.vector.tensor_copy(out=od[:, :S], in_=po[0:64, :S])
                else:
                    # walrus requires matching partition ranges on DVE ops;
                    # odd heads hop through SBUF + DMA to reach offset 64
                    nc.vector.tensor_copy(out=dt_[0:64, :S], in_=po[0:64, :S])
                    nc.sync.dma_start(out=od[:, :S], in_=dt_[0:64, :S])
                nc.sync.dma_start(out=den_d[h][0:1, :], in_=dt_[64:65, :S])

            # interleave: emit the qk tiles of pair p+1 ahead of pair p's heads
            # so the PE has qkv matmuls to chew on while ACT runs exp.
            emit_qk_tile(0)
            emit_qk_tile(ND + 0)
            for p in range(H // 2):
                if p + 1 < H // 2:
                    emit_qk_tile(p + 1)
                    emit_qk_tile(ND + p + 1)
                emit_head(2 * p)
                emit_head(2 * p + 1)

            # batched softmax normalization: 1/den for all heads at once,
            # broadcast back through DRAM, one in-place multiply per outT tile
            den_sb = st.tile([P, S], BF16, name="den_sb", tag="den", bufs=1)
            for h in range(H):
                nc.gpsimd.dma_start(out=den_sb[h:h + 1, :], in_=den_d[h][0:1, :])
            with nc.allow_low_precision(reason="bf16 softmax denom"):
                nc.vector.reciprocal(out=den_sb[:H, :], in_=den_sb[:H, :])
            recip_d = dp.tile([H, S], BF16, name="recip_scr", tag="recip")
            nc.sync.dma_start(out=recip_d[:, :], in_=den_sb[:H, :])
            for j in range(ND):
                rb = st.tile([P, S], BF16, name=f"rb{j}", tag="rb", bufs=2)
                for half in range(2):
                    rsrc = recip_d[2 * j + half:2 * j + half + 1, :]
                    nc.gpsimd.dma_start(
                        out=rb[half * 64:half * 64 + 64, :],
                        in_=bass.AP(tensor=rsrc.tensor, offset=rsrc.offset,
                                    ap=[[0, 64]] + list(rsrc.ap)[1:]))
                nc.vector.tensor_mul(out=outT[j][:, :S], in0=outT[j][:, :S],
                                     in1=rb[:, :S])

            # ---------------- proj + residual + LN2 ----------------
            projw = []
            proj_w3 = io["proj_w"].rearrange("(ko ki) n -> ki ko n", ki=P)
            for ko in range(ND):
                t = wp.tile([P, D], BF16, name=f"projw{ko}", **WT)
                nc.gpsimd.dma_start(out=t[:], in_=proj_w3[:, ko, :])
                projw.append(t)

            psum_mm.i = 1  # last mm1 user was attention; start proj on mm0
            xn2T = [rp.tile([P, S], BF16, name=f"xn2T{j}", **RT)
                    for j in range(ND)]
            for (s0, srows) in stiles:
                pm = psum_mm()
                for ko in range(ND):
                    for ci, (n0, nn) in enumerate(dch):
                        nc.tensor.matmul(pm[:srows, ci * 512:ci * 512 + nn],
                                         lhsT=outT[ko][:, s0:s0 + srows],
                                         rhs=projw[ko][:, n0:n0 + nn],
                                         start=(ko == 0), stop=(ko == ND - 1))
                x_t = st.tile([P, D], F32, name="x_t2", tag="xf", bufs=4)
                nc.gpsimd.dma_start(out=x_t[:srows], in_=x_d[s0:s0 + srows])
                x1_t = st.tile([P, D], F32, name="x1_t", tag="xf", bufs=4)
                nc.vector.tensor_add(out=x1_t[:srows], in0=pm[:srows, 0:D],
                                     in1=x_t[:srows])
                if proj_b is not None:
                    nc.vector.tensor_add(out=x1_t[:srows], in0=x1_t[:srows],
                                         in1=proj_b[:srows])
                nc.sync.dma_start(out=x1_d[s0 // P][:srows], in_=x1_t[:srows])
                xn2_t = layer_norm(x1_t, srows, ln2_g, ln2_b, f"2_{s0}")
                transpose_into(xn2_t, srows, s0, xn2T)

            # ---------------- fc1 -> h^T (gelu fused) ----------------
            fc1_w3 = io["fc1_w"].rearrange("(ko ki) m -> ki ko m", ki=P)
            hT = []
            for mt in range(NM):
                wt = wp.tile([P, ND, P], BF16, name=f"fc1w{mt}", tag="fc1w",
                             bufs=3)
                nc.gpsimd.dma_start(out=wt[:],
                                     in_=fc1_w3[:, :, mt * P:(mt + 1) * P])
                pm = psum_mm()
                for ko in range(ND):
                    for ci, (q0, qn) in enumerate(qch):
                        nc.tensor.matmul(pm[:, ci * 512:ci * 512 + qn],
                                         lhsT=wt[:, ko, :],
                                         rhs=xn2T[ko][:, q0:q0 + qn],
                                         start=(ko == 0), stop=(ko == ND - 1))
                t = rp.tile([P, S], BF16, name=f"hT{mt}", **RT)
                nc.scalar.activation(out=t[:, :S], in_=pm[:, :S],
                                     func=AFT.Gelu if cfg.gelu else AFT.Identity,
                                     bias=fc1_b_sb[:, mt:mt + 1], scale=1.0)
                hT.append(t)

            # ---------------- fc2 + residual ----------------
            fc2w = []
            fc2_w3 = io["fc2_w"].rearrange("(ko ki) n -> ki ko n", ki=P)
            for ko in range(NM):
                t = wp.tile([P, D], BF16, name=f"fc2w{ko}", **WT)
                nc.gpsimd.dma_start(out=t[:], in_=fc2_w3[:, ko, :])
                fc2w.append(t)
            for (s0, srows) in stiles:
                pm = psum_mm()
                for ko in range(NM):
                    for ci, (n0, nn) in enumerate(dch):
                        nc.tensor.matmul(pm[:srows, ci * 512:ci * 512 + nn],
                                         lhsT=hT[ko][:, s0:s0 + srows],
                                         rhs=fc2w[ko][:, n0:n0 + nn],
                                         start=(ko == 0), stop=(ko == NM - 1))
                x1_t = st.tile([P, D], F32, name="x1_t2", tag="xf", bufs=4)
                nc.gpsimd.dma_start(out=x1_t[:srows], in_=x1_d[s0 // P][:srows])
                o_t = st.tile([P, D], F32, name="o_t", tag="xf", bufs=4)
                nc.vector.tensor_add(out=o_t[:srows], in0=pm[:srows, 0:D],
                                     in1=x1_t[:srows])
                if fc2_b is not None:
                    nc.vector.tensor_add(out=o_t[:srows], in0=o_t[:srows],
                                         in1=fc2_b[:srows])
                nc.sync.dma_start(out=out_d[s0:s0 + srows], in_=o_t[:srows])
    return nc


def build_full(cfg):
    nc = bacc.Bacc("TRN2", target_bir_lowering=False, debug=False)
    io = {
        "x": nc.dram_tensor("x", [cfg.S, cfg.D], F32,
                            kind="ExternalInput").ap(),
        "maskT": nc.dram_tensor("maskT", [cfg.NP, cfg.NQ], F32,
                                kind="ExternalInput").ap(),
        "qkv_w": nc.dram_tensor("qkv_w", [cfg.D, 3 * cfg.D], BF16,
                                kind="ExternalInput").ap(),
        "proj_w": nc.dram_tensor("proj_w", [cfg.D, cfg.D], BF16,
                                 kind="ExternalInput").ap(),
        "fc1_w": nc.dram_tensor("fc1_w", [cfg.D, cfg.MLP], BF16,
                                kind="ExternalInput").ap(),
        "fc2_w": nc.dram_tensor("fc2_w", [cfg.MLP, cfg.D], BF16,
                                kind="ExternalInput").ap(),
        "fc1_b": nc.dram_tensor("fc1_b", [cfg.MLP], F32,
                                kind="ExternalInput").ap(),
        "out": nc.dram_tensor("out", [cfg.S, cfg.D], F32,
                              kind="ExternalOutput").ap(),
    }
    for flag, name in [
        (cfg.use_ln1_g, "ln1_g"), (cfg.use_ln1_b, "ln1_b"),
        (cfg.use_ln2_g, "ln2_g"), (cfg.use_ln2_b, "ln2_b"),
        (cfg.use_proj_b, "proj_b"), (cfg.use_fc2_b, "fc2_b"),
    ]:
        if flag:
            io[name] = nc.dram_tensor(name, [cfg.D], F32,
                                      kind="ExternalInput").ap()
    build_layer(nc, cfg, io)
    nc.finalize()  # runs Bacc legalization (wait splitting, regalloc)
    return nc


_CACHE = {}


def kernel(**inputs):
    x = np.asarray(inputs["x"], dtype=np.float32)
    mask = np.asarray(inputs["mask"], dtype=np.float32)
    B, S, D = x.shape
    NQ = mask.shape[1]
    NP = int(np.prod(mask.shape[2:]))
    MLP = inputs["fc1_w"].shape[1]

    cfg = Cfg(
        B=B, S=S, D=D, NP=NP, NQ=NQ, MLP=MLP,
        use_ln1_g=not np.all(np.asarray(inputs["ln1_g"]) == 1.0),
        use_ln1_b=not np.all(np.asarray(inputs["ln1_b"]) == 0.0),
        use_ln2_g=not np.all(np.asarray(inputs["ln2_g"]) == 1.0),
        use_ln2_b=not np.all(np.asarray(inputs["ln2_b"]) == 0.0),
        use_proj_b=not np.all(np.asarray(inputs["proj_b"]) == 0.0),
        use_fc2_b=not np.all(np.asarray(inputs["fc2_b"]) == 0.0),
    )
    key = cfg.key()
    if key not in _CACHE:
        _CACHE[key] = build_full(cfg)
    nc = _CACHE[key]

    bf = ml_dtypes.bfloat16
    shared = {
        "qkv_w": np.ascontiguousarray(np.asarray(inputs["qkv_w"]).astype(bf)),
        "proj_w": np.ascontiguousarray(np.asarray(inputs["proj_w"]).astype(bf)),
        "fc1_w": np.ascontiguousarray(np.asarray(inputs["fc1_w"]).astype(bf)),
        "fc2_w": np.ascontiguousarray(np.asarray(inputs["fc2_w"]).astype(bf)),
        "fc1_b": np.ascontiguousarray(np.asarray(inputs["fc1_b"],
                                                 dtype=np.float32)),
    }
    for flag, name in [(cfg.use_ln1_g, "ln1_g"), (cfg.use_ln1_b, "ln1_b"),
                       (cfg.use_ln2_g, "ln2_g"), (cfg.use_ln2_b, "ln2_b"),
                       (cfg.use_proj_b, "proj_b"), (cfg.use_fc2_b, "fc2_b")]:
        if flag:
            shared[name] = np.ascontiguousarray(
                np.asarray(inputs[name], dtype=np.float32))

    in_maps = []
    for b in range(B):
        m = dict(shared)
        m["x"] = np.ascontiguousarray(x[b])
        m["maskT"] = np.ascontiguousarray(
            mask[b].reshape(NQ, NP).T.astype(np.float32))
        in_maps.append(m)

    from concourse.bass_utils import run_bass_kernel_spmd
    res = run_bass_kernel_spmd(nc, in_maps, core_ids=list(range(B)))
    return np.stack([res.results[b]["out"] for b in range(B)], axis=0)


if __name__ == "__main__":
    cfg = Cfg()
    nc = build_full(cfg)
    print("built ok")



# revision 10
# speedup vs baseline: 1.1091x; 1.1091x over previous
"""Trainium2 Bass kernel for an EoMT transformer encoder layer.

Layer (per batch element):
    xn  = LN1(x);  qkv = xn @ qkv_w;  masked softmax attention (16 heads);
    y   = attn_out @ proj_w + proj_b;  x1 = x + y
    h   = gelu(LN2(x1) @ fc1_w + fc1_b);  y2 = h @ fc2_w + fc2_b; out = x1 + y2

Sharding: pure data-parallel over batch — B=8 maps 1:1 onto the 8 NeuronCores,
no collectives.  Each core runs the full layer for its batch element.

Per-core layout choices:
  - qkv computed in two parts: q,k in TRANSPOSED layout qk^T [2D, S]
    (stationary = qkv_w chunk, moving = xn^T) so per-head q^T,k^T [64, S]
    feed the scores matmul directly; v in NATURAL layout [S, D] (stationary =
    xn^T chunk, moving = w_v) so it is the attn@v stationary directly.
  - scores are computed transposed, scoresT [k_pos, q_pos] = k^T.T @ q^T, so
    the softmax sum over k_pos falls out of a matmul against a ones column
    appended to v (denominator for free), flash-style per k-tile:
    scores -> exp (ACT reads PSUM, writes bf16 SBUF) -> mask-mul -> attn@v.
    No max-subtraction (|score*scale| < ~3 by construction).
  - The TRN2 PE drops to mid p-state (1.2 GHz) whenever it idles, so the
    whole attention phase is emitted as a gapless software pipeline:
    scores matmuls go to a 3-deep ring of 1-bank (512-col) PSUM tiles, exp
    runs per chunk on ACT, attn@v accumulates into a 3-bank po PSUM, and
    the remaining qk^T tiles are interleaved between k-tiles as PE filler
    so the PE queue never starves while ACT works through the exps.
  - out^T [D, S] = v.T @ expT accumulates per head; psum row 64 is the
    denominator.  Normalization multiplies by a DRAM-broadcast reciprocal.
  - LN statistics run on DVE (bn_stats), the normalize itself on ACT
    (Identity activation with per-partition scale/bias), so LN phases
    pipeline across both engines.
  - fc1 emits h^T [MLP, S] with gelu+bias fused into the PSUM->SBUF
    activation; fc2 contracts h^T with fc2_w back to natural [S, D].
    proj/fc1/fc2 all accumulate into the same 3-deep 1-bank PSUM ring in
    512-col chunks.
All matmuls are bf16 (weights pre-cast on host, activations cast on chip),
accumulating fp32 in PSUM.  LN statistics are fp32.
"""

import os
import sys

for _p in ("/opt/trn_rl_repo", "/root/.axon_site/_ro/trn_rl_repo"):
    if _p not in sys.path and os.path.isdir(_p):
        sys.path.append(_p)

import numpy as np
import ml_dtypes

import concourse.bass as bass
import concourse.tile as tile
from concourse import bacc
from concourse import mybir
from concourse.masks import make_identity

AFT = mybir.ActivationFunctionType
ALU = mybir.AluOpType
BF16 = mybir.dt.bfloat16
F32 = mybir.dt.float32

P = 128


class Cfg:
    def __init__(self, B=8, S=1124, D=1024, NP=1024, NQ=100, MLP=4096,
                 EPS=1e-6, use_ln1_g=False, use_ln1_b=False, use_ln2_g=False,
                 use_ln2_b=False, use_proj_b=False, use_fc2_b=False,
                 gelu=True):
        self.B, self.S, self.D = B, S, D
        self.NP, self.NQ, self.MLP, self.EPS = NP, NQ, MLP, EPS
        self.DH = 64
        self.H = D // self.DH
        assert D % P == 0 and MLP % P == 0
        self.SCALE = self.DH ** -0.5
        self.use_ln1_g, self.use_ln1_b = use_ln1_g, use_ln1_b
        self.use_ln2_g, self.use_ln2_b = use_ln2_g, use_ln2_b
        self.use_proj_b, self.use_fc2_b = use_proj_b, use_fc2_b
        self.gelu = gelu

    def key(self):
        return tuple(sorted((k, v) for k, v in self.__dict__.items()))


def _s_tiles(S):
    return [(i * P, min(P, S - i * P)) for i in range((S + P - 1) // P)]


def _chunks(N, width=512):
    return [(i * width, min(width, N - i * width))
            for i in range((N + width - 1) // width)]


def build_layer(nc, cfg, io):
    """Trace the layer program into `nc`.  `io` maps names to DRAM APs."""
    S, D, H, MLP, NP, NQ = cfg.S, cfg.D, cfg.H, cfg.MLP, cfg.NP, cfg.NQ
    ND = D // P                      # contraction chunks of D
    NQK = 2 * D // P                 # m-tiles of transposed q|k
    NM = MLP // P                    # m-tiles of MLP hidden
    stiles = _s_tiles(S)
    NS = len(stiles)
    qch = _chunks(S)                 # free chunks of S, <=512, bank-aligned
    dch = _chunks(D)                 # free chunks of D

    WTW = 1024 if (3 * D) % 1024 == 0 else 3 * D   # qkv weight tile width
    n_qkvw = ND * (3 * D // WTW)
    WBUFS = max(n_qkvw + ND, NM) + 2   # qkv tiles + proj tiles live together

    x_d, out_d, maskT_d = io["x"], io["out"], io["maskT"]

    with tile.TileContext(nc) as tc:
        with (
            tc.tile_pool(name="const", bufs=1) as cpool,
            tc.tile_pool(name="rp", bufs=1) as rp,
            tc.tile_pool(name="wp", bufs=1) as wp,
            tc.tile_pool(name="st", bufs=1) as st,
            tc.tile_pool(name="dp", bufs=1, space="DRAM") as dp,
            tc.tile_pool(name="ps", bufs=1, space="PSUM") as ps,
        ):
            RT = dict(tag="r", bufs=41)
            WT = dict(tag="w", bufs=WBUFS)

            x1_d = [dp.tile([rows, D], F32, name=f"x1_scr{i}", tag=f"x1{i}")
                    for i, (s0, rows) in enumerate(stiles)]
            den_d = [dp.tile([1, S], BF16, name=f"den_scr{h}", tag=f"den{h}")
                     for h in range(H)]

            ident = cpool.tile([P, P], BF16, name="ident")
            make_identity(nc, ident[:])
            eps_t = cpool.tile([P, 1], F32, name="eps")
            nc.vector.memset(eps_t, cfg.EPS)
            negone_t = cpool.tile([P, 1], F32, name="negone")
            nc.vector.memset(negone_t, -1.0)

            # ---- weight DMAs, ordered so attention prerequisites land
            # first: v columns (sync), q columns (vector), k columns (sync),
            # then proj (sync).  fc1/fc2 stream in later.
            qkvw = {}
            qkv_w3 = io["qkv_w"].rearrange("(ko ki) n -> ki ko n", ki=P)
            nwi = 3 * D // WTW
            wi_order = []
            if nwi == 3:
                wi_order = [(2, nc.sync), (0, nc.scalar), (1, nc.sync)]
            else:
                wi_order = [(wi, nc.sync) for wi in range(nwi)]
            for wi, eng in wi_order:
                for ko in range(ND):
                    t = wp.tile([P, WTW], BF16, name=f"qkvw{ko}_{wi}", **WT)
                    eng.dma_start(
                        out=t[:], in_=qkv_w3[:, ko, wi * WTW:(wi + 1) * WTW])
                    qkvw[(ko, wi)] = t

            projw = []
            proj_w3 = io["proj_w"].rearrange("(ko ki) n -> ki ko n", ki=P)
            for ko in range(ND):
                t = wp.tile([P, D], BF16, name=f"projw{ko}", **WT)
                nc.sync.dma_start(out=t[:], in_=proj_w3[:, ko, :])
                projw.append(t)

            def w_col(ko, m0, width):
                wi, off = divmod(m0, WTW)
                assert off + width <= WTW
                return qkvw[(ko, wi)][:, off:off + width]

            def bcast_vec(name, ap_1d):
                t = cpool.tile([P, ap_1d.shape[0]], F32, name=name)
                src = bass.AP(tensor=ap_1d.tensor, offset=ap_1d.offset,
                              ap=[[0, P]] + list(ap_1d.ap))
                nc.sync.dma_start(out=t[:], in_=src)
                return t

            ln1_g = bcast_vec("ln1_g", io["ln1_g"]) if cfg.use_ln1_g else None
            ln1_b = bcast_vec("ln1_b", io["ln1_b"]) if cfg.use_ln1_b else None
            ln2_g = bcast_vec("ln2_g", io["ln2_g"]) if cfg.use_ln2_g else None
            ln2_b = bcast_vec("ln2_b", io["ln2_b"]) if cfg.use_ln2_b else None
            proj_b = bcast_vec("proj_b", io["proj_b"]) if cfg.use_proj_b else None
            fc2_b = bcast_vec("fc2_b", io["fc2_b"]) if cfg.use_fc2_b else None

            fc1_b_sb = cpool.tile([P, NM], F32, name="fc1_b_sb")
            nc.sync.dma_start(out=fc1_b_sb[:],
                              in_=io["fc1_b"].rearrange("(mo ki) -> ki mo", ki=P))

            # binarized transposed mask per (partially) masked k-tile
            mtiles = []
            for kt, (k0, krows) in enumerate(stiles):
                if k0 >= NP or NQ == 0:
                    mtiles.append(None)
                    continue
                mrows = min(k0 + krows, NP) - k0
                mf = st.tile([P, NQ], F32, name=f"mf{kt}", tag="mf", bufs=1)
                nc.gpsimd.dma_start(out=mf[:mrows], in_=maskT_d[k0:k0 + mrows])
                mb = st.tile([P, NQ], BF16, name=f"mb{kt}", tag="mb", bufs=NS)
                nc.vector.tensor_scalar(out=mb[:mrows], in0=mf[:mrows],
                                        scalar1=0.5, scalar2=None,
                                        op0=ALU.is_gt)
                mtiles.append(mb)

            # ---------------- LN + transpose helpers ----------------
            # stats on DVE, normalize on ACT (Identity w/ per-partition
            # scale=rstd, bias=-mean*rstd) so the two engines pipeline.
            def layer_norm(x_t, srows, g, b, name):
                nsub = 2 if D > 512 else 1
                half = D // nsub
                stats = st.tile([P, nsub, 6], F32, name=f"sta{name}",
                                tag="stats", bufs=3)
                mv = st.tile([P, 2], F32, name=f"mv{name}", tag="mv", bufs=3)
                for i in range(nsub):
                    nc.vector.bn_stats(out=stats[:srows, i],
                                       in_=x_t[:srows, i * half:(i + 1) * half])
                nc.vector.bn_aggr(out=mv[:srows], in_=stats[:srows])
                std = st.tile([P, 1], F32, name=f"std{name}", tag="std", bufs=3)
                nc.scalar.activation(out=std[:srows], in_=mv[:srows, 1:2],
                                     func=AFT.Sqrt, bias=eps_t[:srows],
                                     scale=1.0)
                nc.vector.reciprocal(out=std[:srows], in_=std[:srows])
                neg = st.tile([P, 1], F32, name=f"neg{name}", tag="neg", bufs=3)
                nc.vector.tensor_scalar(out=neg[:srows], in0=mv[:srows, 0:1],
                                        scalar1=std[:srows, 0:1],
                                        scalar2=negone_t[:srows],
                                        op0=ALU.mult, op1=ALU.mult)
                xn_t = st.tile([P, D], BF16, name=f"xn{name}", tag="xn", bufs=2)
                nc.scalar.activation(out=xn_t[:srows], in_=x_t[:srows],
                                     func=AFT.Identity, bias=neg[:srows, 0:1],
                                     scale=std[:srows, 0:1])
                if g is not None:
                    nc.vector.tensor_mul(out=xn_t[:srows], in0=xn_t[:srows],
                                         in1=g[:srows])
                if b is not None:
                    nc.vector.tensor_add(out=xn_t[:srows], in0=xn_t[:srows],
                                         in1=b[:srows])
                return xn_t

            def transpose_into(xn_t, srows, s0, dst_tiles):
                # alternate the transpose psum between the 1-bank "tr" slot
                # and the wider "sc" ring so back-to-back transposes pipeline
                for j in range(ND):
                    if j % 2 == 0:
                        pt = ps.tile([P, 512], BF16, name=f"ptr{j}", tag="tr",
                                     bufs=1)
                    else:
                        pt = ps.tile([P, 512], BF16, name=f"psr{j}", tag="sc",
                                     bufs=2)
                    nc.tensor.transpose(pt[:P, :srows],
                                        xn_t[:srows, j * P:(j + 1) * P],
                                        ident[:srows, :srows])
                    nc.vector.tensor_copy(out=dst_tiles[j][:, s0:s0 + srows],
                                          in_=pt[:P, :srows])

            def small_mm(lhsT_fn, rhs_fn, out_rows, chunks, copy_fn):
                """Accumulate over ko into a 1-bank psum per free chunk."""
                for (c0, cn) in chunks:
                    pm = ps.tile([P, 512], F32, name="ptr_mm", tag="tr", bufs=1)
                    for ko in range(ND):
                        nc.tensor.matmul(pm[:out_rows, :cn],
                                         lhsT=lhsT_fn(ko),
                                         rhs=rhs_fn(ko, c0, cn),
                                         start=(ko == 0), stop=(ko == ND - 1))
                    copy_fn(pm, c0, cn)

            # ---------------- LN1 + v, pipelined per s-tile ----------------
            # v natural [S, D] with interleaved ones column (attn@v stationary)
            xnT = [rp.tile([P, S], BF16, name=f"xnT{j}", **RT)
                   for j in range(ND)]
            v_sb = []
            for kt in range(NS):
                t = rp.tile([P, H, 65], BF16, name=f"v{kt}", **RT)
                nc.vector.memset(t[:, :, 64:65], 1.0)
                v_sb.append(t)

            for kt, (s0, srows) in enumerate(stiles):
                x_t = st.tile([P, D], F32, name="x_t", tag="xf", bufs=3)
                nc.gpsimd.dma_start(out=x_t[:srows], in_=x_d[s0:s0 + srows])
                xn_t = layer_norm(x_t, srows, ln1_g, ln1_b, f"1_{s0}")
                transpose_into(xn_t, srows, s0, xnT)

                def vcopy(pm, c0, cn, kt=kt, srows=srows):
                    h0 = c0 // 64
                    nc.vector.tensor_copy(
                        out=v_sb[kt][:srows, h0:h0 + cn // 64, 0:64],
                        in_=pm[:srows, :cn].rearrange("p (h d) -> p h d", d=64))
                small_mm(lambda ko, s0=s0, srows=srows: xnT[ko][:, s0:s0 + srows],
                         lambda ko, c0, cn: w_col(ko, 2 * D + c0, cn),
                         srows, dch, vcopy)

            # ---------------- qk tiles: 0/ND upfront, rest as filler -------
            qkT = [None] * NQK

            def qk_unit(mt, c0, cn):
                """One chunk of one transposed qk tile (a PE filler unit)."""
                def run():
                    t = qkT[mt]
                    pm = ps.tile([P, 512], F32, name="ptr_mm", tag="tr", bufs=1)
                    for ko in range(ND):
                        nc.tensor.matmul(pm[:P, :cn],
                                         lhsT=w_col(ko, mt * P, P),
                                         rhs=xnT[ko][:, c0:c0 + cn],
                                         start=(ko == 0), stop=(ko == ND - 1))
                    nc.vector.tensor_copy(out=t[:, c0:c0 + cn], in_=pm[:P, :cn])
                return run

            def alloc_qk(mt):
                qkT[mt] = rp.tile([P, S], BF16, name=f"qkT{mt}", **RT)

            HP = H // 2
            fillers = []
            for p in range(HP):
                for mt in (p, HP + p):
                    alloc_qk(mt)
            for mt in (0, HP):
                for (c0, cn) in qch:
                    qk_unit(mt, c0, cn)()
            for p in range(1, HP):
                for (c0, cn) in qch:
                    fillers.append(qk_unit(p, c0, cn))
                    fillers.append(qk_unit(HP + p, c0, cn))

            # ---------------- attention: gapless pipeline per head --------
            outT = [rp.tile([P, S], BF16, name=f"outT{j}", **RT)
                    for j in range(ND)]
            den_sb = st.tile([P, S], BF16, name="den_sb", tag="den", bufs=1)

            def emit_head(h):
                qbase, kbase = h * 64, D + h * 64
                qT = qkT[qbase // P][qbase % P:qbase % P + 64, :]
                kT = qkT[kbase // P][kbase % P:kbase % P + 64, :]
                po = ps.tile([P, 1536], F32, name=f"po{h}", tag="po", bufs=1)
                e_t = [None] * NS

                def scores(kt):
                    # q-cols 0:1024 share a 2-bank psum and ONE exp (amortizes
                    # the ~400ns/instr ACT overhead); the 100 query cols go
                    # through the shared 1-bank "tr" slot with their own exp.
                    k0, krows = stiles[kt]
                    e = rp.tile([P, S], BF16, name=f"e{h}_{kt}", tag="expT",
                                bufs=4)
                    pm = ps.tile([P, 1024], F32, name="psc", tag="sc", bufs=2)
                    for (q0, qn) in qch[:2]:
                        nc.tensor.matmul(pm[:krows, q0:q0 + qn],
                                         lhsT=kT[:, k0:k0 + krows],
                                         rhs=qT[:, q0:q0 + qn],
                                         start=True, stop=True)
                    nc.scalar.activation(out=e[:krows, 0:NP],
                                         in_=pm[:krows, 0:NP],
                                         func=AFT.Exp, scale=cfg.SCALE)
                    q0, qn = qch[2]
                    pq = ps.tile([P, 512], F32, name="psq", tag="tr", bufs=1)
                    nc.tensor.matmul(pq[:krows, :qn],
                                     lhsT=kT[:, k0:k0 + krows],
                                     rhs=qT[:, q0:q0 + qn],
                                     start=True, stop=True)
                    nc.scalar.activation(out=e[:krows, q0:q0 + qn],
                                         in_=pq[:krows, :qn],
                                         func=AFT.Exp, scale=cfg.SCALE)
                    if mtiles[kt] is not None:
                        mrows = min(k0 + krows, NP) - k0
                        nc.vector.tensor_mul(out=e[:mrows, NP:S],
                                             in0=e[:mrows, NP:S],
                                             in1=mtiles[kt][:mrows])
                    e_t[kt] = e

                def av(kt):
                    k0, krows = stiles[kt]
                    for ci, (q0, qn) in enumerate(qch):
                        nc.tensor.matmul(po[:65, ci * 512:ci * 512 + qn],
                                         lhsT=v_sb[kt][:krows, h, :],
                                         rhs=e_t[kt][:krows, q0:q0 + qn],
                                         start=(kt == 0), stop=(kt == NS - 1))

                scores(0)
                for kt in range(NS):
                    if kt + 1 < NS:
                        scores(kt + 1)
                    av(kt)
                    if kt % 3 == 1 and fillers:
                        fillers.pop(0)()

                # drain: out rows -> outT strip, denominator row -> den_sb
                off = (h % 2) * 64
                od = outT[h // 2][off:off + 64, :]
                dt_ = st.tile([P, S], BF16, name=f"dt{h}", tag="dt", bufs=2)
                nc.vector.tensor_copy(out=dt_[64:65, :S], in_=po[64:65, :S])
                if off == 0:
                    nc.vector.tensor_copy(out=od[:, :S], in_=po[0:64, :S])
                else:
                    # walrus requires matching partition ranges on DVE ops;
                    # odd heads hop through SBUF + DMA to reach offset 64
                    nc.vector.tensor_copy(out=dt_[0:64, :S], in_=po[0:64, :S])
                    nc.sync.dma_start(out=od[:, :S], in_=dt_[0:64, :S])
                nc.gpsimd.dma_start(out=den_d[h][0:1, :], in_=dt_[64:65, :S])
                nc.gpsimd.dma_start(out=den_sb[h:h + 1, :],
                                    in_=den_d[h][0:1, :])

            for h in range(H):
                emit_head(h)

            # batched softmax normalization: 1/den computed as exp(-ln(den))
            # on the (idle) ACT engine — the DVE reciprocal of [H, S] costs
            # ~8.6us serial; Ln+Exp cost ~2.2us and the f32 intermediate
            # lives in the now-free "po" psum.  Then broadcast via DRAM and
            # one in-place multiply per outT tile.
            po_ln = ps.tile([P, 1536], F32, name="po_ln", tag="po", bufs=1)
            nc.scalar.activation(out=po_ln[:H, :S], in_=den_sb[:H, :S],
                                 func=AFT.Ln, scale=1.0)
            with nc.allow_low_precision(reason="bf16 softmax denom"):
                nc.scalar.activation(out=den_sb[:H, :S], in_=po_ln[:H, :S],
                                     func=AFT.Exp, scale=-1.0)
            recip_d = dp.tile([H, S], BF16, name="recip_scr", tag="recip")
            nc.sync.dma_start(out=recip_d[:, :], in_=den_sb[:H, :])
            for j in range(ND):
                rb = st.tile([P, S], BF16, name=f"rb{j}", tag="rb", bufs=2)
                for half in range(2):
                    rsrc = recip_d[2 * j + half:2 * j + half + 1, :]
                    nc.gpsimd.dma_start(
                        out=rb[half * 64:half * 64 + 64, :],
                        in_=bass.AP(tensor=rsrc.tensor, offset=rsrc.offset,
                                    ap=[[0, 64]] + list(rsrc.ap)[1:]))
                nc.vector.tensor_mul(out=outT[j][:, :S], in0=outT[j][:, :S],
                                     in1=rb[:, :S])

            # ---------------- proj + residual + LN2 ----------------
            xn2T = [rp.tile([P, S], BF16, name=f"xn2T{j}", **RT)
                    for j in range(ND)]
            for (s0, srows) in stiles:
                x_t = st.tile([P, D], F32, name="x_t2", tag="xf", bufs=3)
                nc.gpsimd.dma_start(out=x_t[:srows], in_=x_d[s0:s0 + srows])
                x1_t = st.tile([P, D], F32, name="x1_t", tag="xf", bufs=3)
                for (n0, nn) in dch:
                    pm = ps.tile([P, 512], F32, name="ppr", tag="sc", bufs=2)
                    for ko in range(ND):
                        nc.tensor.matmul(pm[:srows, :nn],
                                         lhsT=outT[ko][:, s0:s0 + srows],
                                         rhs=projw[ko][:, n0:n0 + nn],
                                         start=(ko == 0), stop=(ko == ND - 1))
                    nc.vector.tensor_add(out=x1_t[:srows, n0:n0 + nn],
                                         in0=pm[:srows, :nn],
                                         in1=x_t[:srows, n0:n0 + nn])
                if proj_b is not None:
                    nc.vector.tensor_add(out=x1_t[:srows], in0=x1_t[:srows],
                                         in1=proj_b[:srows])
                nc.sync.dma_start(out=x1_d[s0 // P][:srows], in_=x1_t[:srows])
                xn2_t = layer_norm(x1_t, srows, ln2_g, ln2_b, f"2_{s0}")
                transpose_into(xn2_t, srows, s0, xn2T)

            # fc2 weights prefetch during fc1 compute (sync queue)
            fc2w = []
            fc2_w3 = io["fc2_w"].rearrange("(ko ki) n -> ki ko n", ki=P)
            for ko in range(NM):
                t = wp.tile([P, D], BF16, name=f"fc2w{ko}", **WT)
                nc.sync.dma_start(out=t[:], in_=fc2_w3[:, ko, :])
                fc2w.append(t)

            # ---------------- fc1 -> h^T (gelu fused) ----------------
            fc1_w3 = io["fc1_w"].rearrange("(ko ki) m -> ki ko m", ki=P)
            hT = []
            for mt in range(NM):
                wt = wp.tile([P, ND, P], BF16, name=f"fc1w{mt}", tag="fc1w",
                             bufs=3)
                nc.gpsimd.dma_start(out=wt[:],
                                     in_=fc1_w3[:, :, mt * P:(mt + 1) * P])
                t = rp.tile([P, S], BF16, name=f"hT{mt}", **RT)
                for (q0, qn) in qch:
                    pm = ps.tile([P, 512], F32, name="pfc1", tag="sc", bufs=2)
                    for ko in range(ND):
                        nc.tensor.matmul(pm[:P, :qn],
                                         lhsT=wt[:, ko, :],
                                         rhs=xn2T[ko][:, q0:q0 + qn],
                                         start=(ko == 0), stop=(ko == ND - 1))
                    nc.scalar.activation(out=t[:, q0:q0 + qn], in_=pm[:P, :qn],
                                         func=AFT.Gelu if cfg.gelu
                                         else AFT.Identity,
                                         bias=fc1_b_sb[:, mt:mt + 1],
                                         scale=1.0)
                hT.append(t)

            # ---------------- fc2 + residual ----------------
            for (s0, srows) in stiles:
                x1_t = st.tile([P, D], F32, name="x1_t2", tag="xf", bufs=3)
                nc.gpsimd.dma_start(out=x1_t[:srows], in_=x1_d[s0 // P][:srows])
                o_t = st.tile([P, D], F32, name="o_t", tag="xf", bufs=3)
                for (n0, nn) in dch:
                    pm = ps.tile([P, 512], F32, name="pfc2", tag="sc", bufs=2)
                    for ko in range(NM):
                        nc.tensor.matmul(pm[:srows, :nn],
                                         lhsT=hT[ko][:, s0:s0 + srows],
                                         rhs=fc2w[ko][:, n0:n0 + nn],
                                         start=(ko == 0), stop=(ko == NM - 1))
                    nc.vector.tensor_add(out=o_t[:srows, n0:n0 + nn],
                                         in0=pm[:srows, :nn],
                                         in1=x1_t[:srows, n0:n0 + nn])
                if fc2_b is not None:
                    nc.vector.tensor_add(out=o_t[:srows], in0=o_t[:srows],
                                         in1=fc2_b[:srows])
                nc.sync.dma_start(out=out_d[s0:s0 + srows], in_=o_t[:srows])
    return nc


def build_full(cfg):
    nc = bacc.Bacc("TRN2", target_bir_lowering=False, debug=False)
    io = {
        "x": nc.dram_tensor("x", [cfg.S, cfg.D], F32,
                            kind="ExternalInput").ap(),
        "maskT": nc.dram_tensor("maskT", [cfg.NP, cfg.NQ], F32,
                                kind="ExternalInput").ap(),
        "qkv_w": nc.dram_tensor("qkv_w", [cfg.D, 3 * cfg.D], BF16,
                                kind="ExternalInput").ap(),
        "proj_w": nc.dram_tensor("proj_w", [cfg.D, cfg.D], BF16,
                                 kind="ExternalInput").ap(),
        "fc1_w": nc.dram_tensor("fc1_w", [cfg.D, cfg.MLP], BF16,
                                kind="ExternalInput").ap(),
        "fc2_w": nc.dram_tensor("fc2_w", [cfg.MLP, cfg.D], BF16,
                                kind="ExternalInput").ap(),
        "fc1_b": nc.dram_tensor("fc1_b", [cfg.MLP], F32,
                                kind="ExternalInput").ap(),
        "out": nc.dram_tensor("out", [cfg.S, cfg.D], F32,
                              kind="ExternalOutput").ap(),
    }
    for flag, name in [
        (cfg.use_ln1_g, "ln1_g"), (cfg.use_ln1_b, "ln1_b"),
        (cfg.use_ln2_g, "ln2_g"), (cfg.use_ln2_b, "ln2_b"),
        (cfg.use_proj_b, "proj_b"), (cfg.use_fc2_b, "fc2_b"),
    ]:
        if flag:
            io[name] = nc.dram_tensor(name, [cfg.D], F32,
                                      kind="ExternalInput").ap()
    build_layer(nc, cfg, io)
    nc.finalize()  # runs Bacc legalization (wait splitting, regalloc)
    return nc


_CACHE = {}


def kernel(**inputs):
    x = np.asarray(inputs["x"], dtype=np.float32)
    mask = np.asarray(inputs["mask"], dtype=np.float32)
    B, S, D = x.shape
    NQ = mask.shape[1]
    NP = int(np.prod(mask.shape[2:]))
    MLP = inputs["fc1_w"].shape[1]

    cfg = Cfg(
        B=B, S=S, D=D, NP=NP, NQ=NQ, MLP=MLP,
        use_ln1_g=not np.all(np.asarray(inputs["ln1_g"]) == 1.0),
        use_ln1_b=not np.all(np.asarray(inputs["ln1_b"]) == 0.0),
        use_ln2_g=not np.all(np.asarray(inputs["ln2_g"]) == 1.0),
        use_ln2_b=not np.all(np.asarray(inputs["ln2_b"]) == 0.0),
        use_proj_b=not np.all(np.asarray(inputs["proj_b"]) == 0.0),
        use_fc2_b=not np.all(np.asarray(inputs["fc2_b"]) == 0.0),
    )
    key = cfg.key()
    if key not in _CACHE:
        _CACHE[key] = build_full(cfg)
    nc = _CACHE[key]

    bf = ml_dtypes.bfloat16
    shared = {
        "qkv_w": np.ascontiguousarray(np.asarray(inputs["qkv_w"]).astype(bf)),
        "proj_w": np.ascontiguousarray(np.asarray(inputs["proj_w"]).astype(bf)),
        "fc1_w": np.ascontiguousarray(np.asarray(inputs["fc1_w"]).astype(bf)),
        "fc2_w": np.ascontiguousarray(np.asarray(inputs["fc2_w"]).astype(bf)),
        "fc1_b": np.ascontiguousarray(np.asarray(inputs["fc1_b"],
                                                 dtype=np.float32)),
    }
    for flag, name in [(cfg.use_ln1_g, "ln1_g"), (cfg.use_ln1_b, "ln1_b"),
                       (cfg.use_ln2_g, "ln2_g"), (cfg.use_ln2_b, "ln2_b"),
                       (cfg.use_proj_b, "proj_b"), (cfg.use_fc2_b, "fc2_b")]:
        if flag:
            shared[name] = np.ascontiguousarray(
                np.asarray(inputs[name], dtype=np.float32))

    in_maps = []
    for b in range(B):
        m = dict(shared)
        m["x"] = np.ascontiguousarray(x[b])
        m["maskT"] = np.ascontiguousarray(
            mask[b].reshape(NQ, NP).T.astype(np.float32))
        in_maps.append(m)

    from concourse.bass_utils import run_bass_kernel_spmd
    res = run_bass_kernel_spmd(nc, in_maps, core_ids=list(range(B)))
    return np.stack([res.results[b]["out"] for b in range(B)], axis=0)


if __name__ == "__main__":
    cfg = Cfg()
    nc = build_full(cfg)
    print("built ok")


# revision 15
# speedup vs baseline: 1.2141x; 1.0947x over previous
"""Trainium2 Bass kernel for an EoMT transformer encoder layer.

Layer (per batch element):
    xn  = LN1(x);  qkv = xn @ qkv_w;  masked softmax attention (16 heads);
    y   = attn_out @ proj_w + proj_b;  x1 = x + y
    h   = gelu(LN2(x1) @ fc1_w + fc1_b);  y2 = h @ fc2_w + fc2_b; out = x1 + y2

Sharding: pure data-parallel over batch — B=8 maps 1:1 onto the 8 NeuronCores,
no collectives.  Each core runs the full layer for its batch element.

Per-core layout choices:
  - qkv computed in two parts: q,k in TRANSPOSED layout qk^T [2D, S]
    (stationary = qkv_w chunk, moving = xn^T) so per-head q^T,k^T [64, S]
    feed the scores matmul directly; v in NATURAL layout [S, D] (stationary =
    xn^T chunk, moving = w_v) so it is the attn@v stationary directly.
  - scores are computed transposed, scoresT [k_pos, q_pos] = k^T.T @ q^T, so
    the softmax sum over k_pos falls out of a matmul against a ones column
    appended to v (denominator for free), flash-style per k-tile:
    scores -> exp (ACT reads PSUM, writes bf16 SBUF) -> mask-mul -> attn@v.
    No max-subtraction (|score*scale| < ~3 by construction).
  - The TRN2 PE drops to mid p-state (1.2 GHz) whenever it idles, so the
    whole attention phase is emitted as a gapless software pipeline:
    scores matmuls go to a 3-deep ring of 1-bank (512-col) PSUM tiles, exp
    runs per chunk on ACT, attn@v accumulates into a 3-bank po PSUM, and
    the remaining qk^T tiles are interleaved between k-tiles as PE filler
    so the PE queue never starves while ACT works through the exps.
  - out^T [D, S] = v.T @ expT accumulates per head; psum row 64 is the
    denominator.  Normalization multiplies by a DRAM-broadcast reciprocal.
  - LN statistics run on DVE (bn_stats), the normalize itself on ACT
    (Identity activation with per-partition scale/bias), so LN phases
    pipeline across both engines.
  - fc1 emits h^T [MLP, S] with gelu+bias fused into the PSUM->SBUF
    activation; fc2 contracts h^T with fc2_w back to natural [S, D].
    proj/fc1/fc2 all accumulate into the same 3-deep 1-bank PSUM ring in
    512-col chunks.
All matmuls are bf16 (weights pre-cast on host, activations cast on chip),
accumulating fp32 in PSUM.  LN statistics are fp32.
"""

import os
import sys

for _p in ("/opt/trn_rl_repo", "/root/.axon_site/_ro/trn_rl_repo"):
    if _p not in sys.path and os.path.isdir(_p):
        sys.path.append(_p)

import numpy as np
import ml_dtypes

import concourse.bass as bass
import concourse.tile as tile
from concourse import bacc
from concourse import mybir
from concourse.masks import make_identity

AFT = mybir.ActivationFunctionType
ALU = mybir.AluOpType
BF16 = mybir.dt.bfloat16
F32 = mybir.dt.float32
F8 = mybir.dt.float8e4

P = 128


class Cfg:
    def __init__(self, B=8, S=1124, D=1024, NP=1024, NQ=100, MLP=4096,
                 EPS=1e-6, use_ln1_g=False, use_ln1_b=False, use_ln2_g=False,
                 use_ln2_b=False, use_proj_b=False, use_fc2_b=False,
                 gelu=True):
        self.B, self.S, self.D = B, S, D
        self.NP, self.NQ, self.MLP, self.EPS = NP, NQ, MLP, EPS
        self.DH = 64
        self.H = D // self.DH
        assert D % P == 0 and MLP % P == 0
        self.SCALE = self.DH ** -0.5
        self.use_ln1_g, self.use_ln1_b = use_ln1_g, use_ln1_b
        self.use_ln2_g, self.use_ln2_b = use_ln2_g, use_ln2_b
        self.use_proj_b, self.use_fc2_b = use_proj_b, use_fc2_b
        self.gelu = gelu

    def key(self):
        return tuple(sorted((k, v) for k, v in self.__dict__.items()))


def _s_tiles(S):
    return [(i * P, min(P, S - i * P)) for i in range((S + P - 1) // P)]


def _chunks(N, width=512):
    return [(i * width, min(width, N - i * width))
            for i in range((N + width - 1) // width)]


def build_layer(nc, cfg, io):
    """Trace the layer program into `nc`.  `io` maps names to DRAM APs."""
    S, D, H, MLP, NP, NQ = cfg.S, cfg.D, cfg.H, cfg.MLP, cfg.NP, cfg.NQ
    ND = D // P                      # contraction chunks of D
    NQK = 2 * D // P                 # m-tiles of transposed q|k
    NM = MLP // P                    # m-tiles of MLP hidden
    stiles = _s_tiles(S)
    NS = len(stiles)
    qch = _chunks(S)                 # free chunks of S, <=512, bank-aligned
    dch = _chunks(D)                 # free chunks of D

    WTW = 1024 if (3 * D) % 1024 == 0 else 3 * D   # qkv weight tile width
    n_qkvw = ND * (3 * D // WTW)
    WBUFS = max(n_qkvw + ND, NM) + 2   # qkv tiles + proj tiles live together

    x_d, out_d, maskT_d = io["x"], io["out"], io["maskT"]

    with tile.TileContext(nc) as tc:
        with (
            tc.tile_pool(name="const", bufs=1) as cpool,
            tc.tile_pool(name="rp", bufs=1) as rp,
            tc.tile_pool(name="wp", bufs=1) as wp,
            tc.tile_pool(name="st", bufs=1) as st,
            tc.tile_pool(name="dp", bufs=1, space="DRAM") as dp,
            tc.tile_pool(name="ps", bufs=1, space="PSUM") as ps,
        ):
            RT = dict(tag="r", bufs=41)
            WT = dict(tag="w", bufs=WBUFS)

            x1_d = [dp.tile([rows, D], F32, name=f"x1_scr{i}", tag=f"x1{i}")
                    for i, (s0, rows) in enumerate(stiles)]
            den_d = [dp.tile([1, S], BF16, name=f"den_scr{h}", tag=f"den{h}")
                     for h in range(H)]

            ident = cpool.tile([P, P], BF16, name="ident")
            make_identity(nc, ident[:])
            ident8 = cpool.tile([P, P], F8, name="ident8")
            nc.vector.tensor_copy(out=ident8[:], in_=ident[:])
            eps_t = cpool.tile([P, 1], F32, name="eps")
            nc.vector.memset(eps_t, cfg.EPS)
            negone_t = cpool.tile([P, 1], F32, name="negone")
            nc.vector.memset(negone_t, -1.0)

            # ---- x tiles first on the gpsimd queue so LN1 is not starved
            # by the weight-DMA flood sharing the DMA engine pool
            x_pre = {}
            for kt in range(min(3, NS)):
                s0, srows = stiles[kt]
                xt = st.tile([P, D], F32, name=f"x_pre{kt}", tag="xf", bufs=3)
                nc.gpsimd.dma_start(out=xt[:srows], in_=x_d[s0:s0 + srows])
                x_pre[kt] = xt

            # ---- weight DMAs, ordered so attention prerequisites land
            # first: v columns (sync), q columns (vector), k columns (sync),
            # then proj (sync).  fc1/fc2 stream in later.
            qkvw = {}
            qkv_w3 = io["qkv_w"].rearrange("(ko ki) n -> ki ko n", ki=P)
            nwi = 3 * D // WTW
            wi_order = []
            if nwi == 3:
                wi_order = [(2, nc.sync), (0, nc.sync), (1, nc.sync)]
            else:
                wi_order = [(wi, nc.sync) for wi in range(nwi)]
            for wi, eng in wi_order:
                for ko in range(ND):
                    t = wp.tile([P, WTW], BF16, name=f"qkvw{ko}_{wi}", **WT)
                    eng.dma_start(
                        out=t[:], in_=qkv_w3[:, ko, wi * WTW:(wi + 1) * WTW])
                    qkvw[(ko, wi)] = t

            projw = []
            proj_w3 = io["proj_w"].rearrange("(ko ki) n -> ki ko n", ki=P)
            for ko in range(ND):
                t = wp.tile([P, D], BF16, name=f"projw{ko}", **WT)
                nc.sync.dma_start(out=t[:], in_=proj_w3[:, ko, :])
                projw.append(t)

            def w_col(ko, m0, width):
                wi, off = divmod(m0, WTW)
                assert off + width <= WTW
                return qkvw[(ko, wi)][:, off:off + width]

            def bcast_vec(name, ap_1d):
                t = cpool.tile([P, ap_1d.shape[0]], F32, name=name)
                src = bass.AP(tensor=ap_1d.tensor, offset=ap_1d.offset,
                              ap=[[0, P]] + list(ap_1d.ap))
                nc.sync.dma_start(out=t[:], in_=src)
                return t

            ln1_g = bcast_vec("ln1_g", io["ln1_g"]) if cfg.use_ln1_g else None
            ln1_b = bcast_vec("ln1_b", io["ln1_b"]) if cfg.use_ln1_b else None
            ln2_g = bcast_vec("ln2_g", io["ln2_g"]) if cfg.use_ln2_g else None
            ln2_b = bcast_vec("ln2_b", io["ln2_b"]) if cfg.use_ln2_b else None
            proj_b = bcast_vec("proj_b", io["proj_b"]) if cfg.use_proj_b else None
            fc2_b = bcast_vec("fc2_b", io["fc2_b"]) if cfg.use_fc2_b else None

            fc1_b_sb = cpool.tile([P, NM], F32, name="fc1_b_sb")
            nc.sync.dma_start(out=fc1_b_sb[:],
                              in_=io["fc1_b"].rearrange("(mo ki) -> ki mo", ki=P))


            # ---------------- LN + transpose helpers ----------------
            # stats on DVE, normalize on ACT (Identity w/ per-partition
            # scale=rstd, bias=-mean*rstd) so the two engines pipeline.
            def layer_norm(x_t, srows, g, b, name, out_dtype=BF16):
                nsub = 2 if D > 512 else 1
                half = D // nsub
                stats = st.tile([P, nsub, 6], F32, name=f"sta{name}",
                                tag="stats", bufs=3)
                mv = st.tile([P, 2], F32, name=f"mv{name}", tag="mv", bufs=3)
                for i in range(nsub):
                    nc.vector.bn_stats(out=stats[:srows, i],
                                       in_=x_t[:srows, i * half:(i + 1) * half])
                nc.vector.bn_aggr(out=mv[:srows], in_=stats[:srows])
                std = st.tile([P, 1], F32, name=f"std{name}", tag="std", bufs=3)
                nc.scalar.activation(out=std[:srows], in_=mv[:srows, 1:2],
                                     func=AFT.Sqrt, bias=eps_t[:srows],
                                     scale=1.0)
                nc.vector.reciprocal(out=std[:srows], in_=std[:srows])
                neg = st.tile([P, 1], F32, name=f"neg{name}", tag="neg", bufs=3)
                nc.vector.tensor_scalar(out=neg[:srows], in0=mv[:srows, 0:1],
                                        scalar1=std[:srows, 0:1],
                                        scalar2=negone_t[:srows],
                                        op0=ALU.mult, op1=ALU.mult)
                xn_t = st.tile([P, D], out_dtype, name=f"xn{name}",
                               tag="xn", bufs=2)
                nc.scalar.activation(out=xn_t[:srows], in_=x_t[:srows],
                                     func=AFT.Identity, bias=neg[:srows, 0:1],
                                     scale=std[:srows, 0:1])
                if g is not None:
                    nc.vector.tensor_mul(out=xn_t[:srows], in0=xn_t[:srows],
                                         in1=g[:srows])
                if b is not None:
                    nc.vector.tensor_add(out=xn_t[:srows], in0=xn_t[:srows],
                                         in1=b[:srows])
                return xn_t

            def transpose_into(xn_t, srows, s0, write, dtype=BF16):
                # alternate the transpose psum between the 1-bank "tr" slot
                # and the wider "sc" ring so back-to-back transposes pipeline
                for j in range(ND):
                    if j % 2 == 0:
                        pt = ps.tile([P, 512], dtype, name=f"ptr{j}", tag="tr",
                                     bufs=1)
                    else:
                        pt = ps.tile([P, 512], dtype, name=f"psr{j}", tag="sc",
                                     bufs=2)
                    idn = ident8 if dtype == F8 else ident
                    nc.tensor.transpose(pt[:P, :srows],
                                        xn_t[:srows, j * P:(j + 1) * P],
                                        idn[:srows, :srows])
                    write(j, pt[:P, :srows])

            def small_mm(lhsT_fn, rhs_fn, out_rows, chunks, copy_fn):
                """Accumulate over ko into a 1-bank psum per free chunk."""
                for (c0, cn) in chunks:
                    pm = ps.tile([P, 512], F32, name="ptr_mm", tag="tr", bufs=1)
                    for ko in range(ND):
                        nc.tensor.matmul(pm[:out_rows, :cn],
                                         lhsT=lhsT_fn(ko),
                                         rhs=rhs_fn(ko, c0, cn),
                                         start=(ko == 0), stop=(ko == ND - 1))
                    copy_fn(pm, c0, cn)

            # ---------------- LN1 + v, pipelined per s-tile ----------------
            # v natural [S, D] with interleaved ones column (attn@v stationary)
            xnT = [rp.tile([P, S], BF16, name=f"xnT{j}", **RT)
                   for j in range(ND)]
            v_sb = []
            for kt in range(NS):
                t = rp.tile([P, H, 65], BF16, name=f"v{kt}", **RT)
                nc.vector.memset(t[:, :, 64:65], 1.0)
                v_sb.append(t)

            for kt, (s0, srows) in enumerate(stiles):
                if kt in x_pre:
                    x_t = x_pre[kt]
                else:
                    x_t = st.tile([P, D], F32, name="x_t", tag="xf", bufs=3)
                    nc.gpsimd.dma_start(out=x_t[:srows],
                                        in_=x_d[s0:s0 + srows])
                xn_t = layer_norm(x_t, srows, ln1_g, ln1_b, f"1_{s0}")

                def wr1(j, src_ap, s0=s0, srows=srows):
                    nc.vector.tensor_copy(out=xnT[j][:, s0:s0 + srows],
                                          in_=src_ap)
                transpose_into(xn_t, srows, s0, wr1)

                def vcopy(pm, c0, cn, kt=kt, srows=srows):
                    h0 = c0 // 64
                    nc.vector.tensor_copy(
                        out=v_sb[kt][:srows, h0:h0 + cn // 64, 0:64],
                        in_=pm[:srows, :cn].rearrange("p (h d) -> p h d", d=64))
                small_mm(lambda ko, s0=s0, srows=srows: xnT[ko][:, s0:s0 + srows],
                         lambda ko, c0, cn: w_col(ko, 2 * D + c0, cn),
                         srows, dch, vcopy)

            # binarized transposed mask per (partially) masked k-tile
            # (emitted after LN1 so the mask DMAs + binarize don't block the
            # LN chain on the gpsimd/DVE queues at startup)
            mtiles = []
            for kt, (k0, krows) in enumerate(stiles):
                if k0 >= NP or NQ == 0:
                    mtiles.append(None)
                    continue
                mrows = min(k0 + krows, NP) - k0
                mf = st.tile([P, NQ], F32, name=f"mf{kt}", tag="mf", bufs=1)
                nc.gpsimd.dma_start(out=mf[:mrows], in_=maskT_d[k0:k0 + mrows])
                mb = st.tile([P, NQ], BF16, name=f"mb{kt}", tag="mb", bufs=NS)
                nc.vector.tensor_scalar(out=mb[:mrows], in0=mf[:mrows],
                                        scalar1=0.5, scalar2=None,
                                        op0=ALU.is_gt)
                mtiles.append(mb)

            # ---------------- qk tiles: 0/ND upfront, rest as filler -------
            qkT = [None] * NQK

            def qk_unit(mt, c0, cn):
                """One chunk of one transposed qk tile (a PE filler unit)."""
                def run():
                    t = qkT[mt]
                    pm = ps.tile([P, 512], F32, name="ptr_mm", tag="tr", bufs=1)
                    for ko in range(ND):
                        nc.tensor.matmul(pm[:P, :cn],
                                         lhsT=w_col(ko, mt * P, P),
                                         rhs=xnT[ko][:, c0:c0 + cn],
                                         start=(ko == 0), stop=(ko == ND - 1))
                    nc.vector.tensor_copy(out=t[:, c0:c0 + cn], in_=pm[:P, :cn])
                return run

            def alloc_qk(mt):
                qkT[mt] = rp.tile([P, S], BF16, name=f"qkT{mt}", **RT)

            HP = H // 2
            fillers = []
            for p in range(HP):
                for mt in (p, HP + p):
                    alloc_qk(mt)
            for mt in (0, HP):
                for (c0, cn) in qch:
                    qk_unit(mt, c0, cn)()
            for p in range(1, HP):
                for (c0, cn) in qch:
                    fillers.append(qk_unit(p, c0, cn))
                    fillers.append(qk_unit(HP + p, c0, cn))

            # ---------------- attention: gapless pipeline per head --------
            outT = [rp.tile([P, S], BF16, name=f"outT{j}", **RT)
                    for j in range(ND)]
            den_sb = st.tile([P, S], BF16, name="den_sb", tag="den", bufs=1)

            def emit_head(h):
                qbase, kbase = h * 64, D + h * 64
                qT = qkT[qbase // P][qbase % P:qbase % P + 64, :]
                kT = qkT[kbase // P][kbase % P:kbase % P + 64, :]
                po = ps.tile([P, 1536], F32, name=f"po{h}", tag="po", bufs=1)
                e_t = [None] * NS

                def scores(kt):
                    # q-cols 0:1024 share a 2-bank psum and ONE exp (amortizes
                    # the ~400ns/instr ACT overhead); the 100 query cols go
                    # through the shared 1-bank "tr" slot with their own exp.
                    k0, krows = stiles[kt]
                    e = rp.tile([P, S], BF16, name=f"e{h}_{kt}", tag="expT",
                                bufs=4)
                    pm = ps.tile([P, 1024], F32, name="psc", tag="sc", bufs=2)
                    for (q0, qn) in qch[:2]:
                        nc.tensor.matmul(pm[:krows, q0:q0 + qn],
                                         lhsT=kT[:, k0:k0 + krows],
                                         rhs=qT[:, q0:q0 + qn],
                                         start=True, stop=True)
                    nc.scalar.activation(out=e[:krows, 0:NP],
                                         in_=pm[:krows, 0:NP],
                                         func=AFT.Exp, scale=cfg.SCALE)
                    q0, qn = qch[2]
                    pq = ps.tile([P, 512], F32, name="psq", tag="tr", bufs=1)
                    nc.tensor.matmul(pq[:krows, :qn],
                                     lhsT=kT[:, k0:k0 + krows],
                                     rhs=qT[:, q0:q0 + qn],
                                     start=True, stop=True)
                    nc.scalar.activation(out=e[:krows, q0:q0 + qn],
                                         in_=pq[:krows, :qn],
                                         func=AFT.Exp, scale=cfg.SCALE)
                    if mtiles[kt] is not None:
                        mrows = min(k0 + krows, NP) - k0
                        nc.vector.tensor_mul(out=e[:mrows, NP:S],
                                             in0=e[:mrows, NP:S],
                                             in1=mtiles[kt][:mrows])
                    e_t[kt] = e

                def av(kt):
                    k0, krows = stiles[kt]
                    for ci, (q0, qn) in enumerate(qch):
                        nc.tensor.matmul(po[:65, ci * 512:ci * 512 + qn],
                                         lhsT=v_sb[kt][:krows, h, :],
                                         rhs=e_t[kt][:krows, q0:q0 + qn],
                                         start=(kt == 0), stop=(kt == NS - 1))

                scores(0)
                for kt in range(NS):
                    if kt + 1 < NS:
                        scores(kt + 1)
                    av(kt)
                    if kt % 3 == 1 and fillers:
                        fillers.pop(0)()

                # drain: out rows -> outT strip, denominator row -> den_sb
                off = (h % 2) * 64
                od = outT[h // 2][off:off + 64, :]
                dt_ = st.tile([P, S], BF16, name=f"dt{h}", tag="dt", bufs=2)
                nc.vector.tensor_copy(out=dt_[64:65, :S], in_=po[64:65, :S])
                if off == 0:
                    nc.vector.tensor_copy(out=od[:, :S], in_=po[0:64, :S])
                else:
                    # walrus requires matching partition ranges on DVE ops;
                    # odd heads hop through SBUF + DMA to reach offset 64
                    nc.vector.tensor_copy(out=dt_[0:64, :S], in_=po[0:64, :S])
                    nc.sync.dma_start(out=od[:, :S], in_=dt_[0:64, :S])
                nc.gpsimd.dma_start(out=den_d[h][0:1, :], in_=dt_[64:65, :S])
                nc.gpsimd.dma_start(out=den_sb[h:h + 1, :],
                                    in_=den_d[h][0:1, :])

            for h in range(H):
                emit_head(h)

            # batched softmax normalization: 1/den computed as exp(-ln(den))
            # on the (idle) ACT engine — the DVE reciprocal of [H, S] costs
            # ~8.6us serial; Ln+Exp cost ~2.2us and the f32 intermediate
            # lives in the now-free "po" psum.  Then broadcast via DRAM and
            # one in-place multiply per outT tile.
            po_ln = ps.tile([P, 1536], F32, name="po_ln", tag="po", bufs=1)
            nc.scalar.activation(out=po_ln[:H, :S], in_=den_sb[:H, :S],
                                 func=AFT.Ln, scale=1.0)
            with nc.allow_low_precision(reason="bf16 softmax denom"):
                nc.scalar.activation(out=den_sb[:H, :S], in_=po_ln[:H, :S],
                                     func=AFT.Exp, scale=-1.0)
            recip_d = dp.tile([H, S], BF16, name="recip_scr", tag="recip")
            nc.sync.dma_start(out=recip_d[:, :], in_=den_sb[:H, :])
            for j in range(ND):
                rb = st.tile([P, S], BF16, name=f"rb{j}", tag="rb", bufs=2)
                for half in range(2):
                    rsrc = recip_d[2 * j + half:2 * j + half + 1, :]
                    eng = nc.gpsimd if half == 0 else nc.sync
                    eng.dma_start(
                        out=rb[half * 64:half * 64 + 64, :],
                        in_=bass.AP(tensor=rsrc.tensor, offset=rsrc.offset,
                                    ap=[[0, 64]] + list(rsrc.ap)[1:]))
                nc.vector.tensor_mul(out=outT[j][:, :S], in0=outT[j][:, :S],
                                     in1=rb[:, :S])

            # ---------------- proj + residual + LN2 ----------------
            # xn2^T in fp8 pair-tiles [P, 2, S] (ko-pairs adjacent on dim1)
            # feeding the fc1 DoubleRow matmuls; same RT slot size as bf16
            xn2T8 = [rp.tile([P, 2, S], F8, name=f"xn2T8_{j}", **RT)
                     for j in range(ND // 2)]
            for (s0, srows) in stiles:
                x_t = st.tile([P, D], F32, name="x_t2", tag="xf", bufs=3)
                nc.gpsimd.dma_start(out=x_t[:srows], in_=x_d[s0:s0 + srows])
                x1_t = st.tile([P, D], F32, name="x1_t", tag="xf", bufs=3)
                for (n0, nn) in dch:
                    pm = ps.tile([P, 512], F32, name="ppr", tag="sc", bufs=2)
                    for ko in range(ND):
                        nc.tensor.matmul(pm[:srows, :nn],
                                         lhsT=outT[ko][:, s0:s0 + srows],
                                         rhs=projw[ko][:, n0:n0 + nn],
                                         start=(ko == 0), stop=(ko == ND - 1))
                    nc.vector.tensor_add(out=x1_t[:srows, n0:n0 + nn],
                                         in0=pm[:srows, :nn],
                                         in1=x_t[:srows, n0:n0 + nn])
                if proj_b is not None:
                    nc.vector.tensor_add(out=x1_t[:srows], in0=x1_t[:srows],
                                         in1=proj_b[:srows])
                nc.sync.dma_start(out=x1_d[s0 // P][:srows], in_=x1_t[:srows])
                xn2_t = layer_norm(x1_t, srows, ln2_g, ln2_b, f"2_{s0}")

                def wr2(j, src_ap, s0=s0, srows=srows):
                    # bf16 psum -> fp8 SBUF cast happens in the copy (the
                    # fp8 PE-transpose path needs strided outputs, so the
                    # transpose itself stays bf16)
                    nc.vector.tensor_copy(
                        out=xn2T8[j // 2][:, j % 2, s0:s0 + srows], in_=src_ap)
                transpose_into(xn2_t, srows, s0, wr2)

            # fc2 weights prefetch during fc1 compute (sync queue)
            fc2w = []
            fc2_w3 = io["fc2_w"].rearrange("(ko ki) n -> ki ko n", ki=P)
            for ko in range(NM):
                t = wp.tile([P, D], BF16, name=f"fc2w{ko}", **WT)
                nc.sync.dma_start(out=t[:], in_=fc2_w3[:, ko, :])
                fc2w.append(t)

            # ---------------- fc1 -> h^T (gelu fused) ----------------
            # fc1 in fp8 DoubleRow: weights are pre-scaled x64 on the host
            # (centers w*0.02 into e4m3 range); the 1/64 is folded into the
            # gelu activation scale.  Each matmul contracts a ko-PAIR (256).
            fc1_w3 = io["fc1_w"].rearrange("(ko ki) m -> ki ko m", ki=P)
            NKP = ND // 2
            hT = []
            for mt in range(NM):
                wt = wp.tile([P, ND, P], mybir.dt.uint8, name=f"fc1w{mt}",
                             tag="fc1w", bufs=3)
                nc.gpsimd.dma_start(out=wt[:],
                                     in_=fc1_w3[:, :, mt * P:(mt + 1) * P])
                t = rp.tile([P, S], BF16, name=f"hT{mt}", **RT)
                for (q0, qn) in qch:
                    pm = ps.tile([P, 512], F32, name="pfc1", tag="sc", bufs=2)
                    for kp in range(NKP):
                        nc.tensor.matmul(
                            pm[:P, :qn],
                            lhsT=wt[:, 2 * kp:2 * kp + 2, :].bitcast(F8),
                            rhs=xn2T8[kp][:, :, q0:q0 + qn],
                            start=(kp == 0), stop=(kp == NKP - 1),
                            perf_mode=mybir.MatmulPerfMode.DoubleRow)
                    nc.scalar.activation(out=t[:, q0:q0 + qn], in_=pm[:P, :qn],
                                         func=AFT.Gelu if cfg.gelu
                                         else AFT.Identity,
                                         bias=fc1_b_sb[:, mt:mt + 1],
                                         scale=1.0 / 64.0)
                hT.append(t)

            # ---------------- fc2 + residual ----------------
            for (s0, srows) in stiles:
                x1_t = st.tile([P, D], F32, name="x1_t2", tag="xf", bufs=3)
                nc.gpsimd.dma_start(out=x1_t[:srows], in_=x1_d[s0 // P][:srows])
                o_t = st.tile([P, D], F32, name="o_t", tag="xf", bufs=3)
                for (n0, nn) in dch:
                    pm = ps.tile([P, 512], F32, name="pfc2", tag="sc", bufs=2)
                    for ko in range(NM):
                        nc.tensor.matmul(pm[:srows, :nn],
                                         lhsT=hT[ko][:, s0:s0 + srows],
                                         rhs=fc2w[ko][:, n0:n0 + nn],
                                         start=(ko == 0), stop=(ko == NM - 1))
                    nc.vector.tensor_add(out=o_t[:srows, n0:n0 + nn],
                                         in0=pm[:srows, :nn],
                                         in1=x1_t[:srows, n0:n0 + nn])
                if fc2_b is not None:
                    nc.vector.tensor_add(out=o_t[:srows], in0=o_t[:srows],
                                         in1=fc2_b[:srows])
                nc.sync.dma_start(out=out_d[s0:s0 + srows], in_=o_t[:srows])
    return nc


def build_full(cfg):
    nc = bacc.Bacc("TRN2", target_bir_lowering=False, debug=False)
    io = {
        "x": nc.dram_tensor("x", [cfg.S, cfg.D], F32,
                            kind="ExternalInput").ap(),
        "maskT": nc.dram_tensor("maskT", [cfg.NP, cfg.NQ], F32,
                                kind="ExternalInput").ap(),
        "qkv_w": nc.dram_tensor("qkv_w", [cfg.D, 3 * cfg.D], BF16,
                                kind="ExternalInput").ap(),
        "proj_w": nc.dram_tensor("proj_w", [cfg.D, cfg.D], BF16,
                                 kind="ExternalInput").ap(),
        "fc1_w": nc.dram_tensor("fc1_w", [cfg.D, cfg.MLP],
                                mybir.dt.uint8,
                                kind="ExternalInput").ap(),
        "fc2_w": nc.dram_tensor("fc2_w", [cfg.MLP, cfg.D], BF16,
                                kind="ExternalInput").ap(),
        "fc1_b": nc.dram_tensor("fc1_b", [cfg.MLP], F32,
                                kind="ExternalInput").ap(),
        "out": nc.dram_tensor("out", [cfg.S, cfg.D], F32,
                              kind="ExternalOutput").ap(),
    }
    for flag, name in [
        (cfg.use_ln1_g, "ln1_g"), (cfg.use_ln1_b, "ln1_b"),
        (cfg.use_ln2_g, "ln2_g"), (cfg.use_ln2_b, "ln2_b"),
        (cfg.use_proj_b, "proj_b"), (cfg.use_fc2_b, "fc2_b"),
    ]:
        if flag:
            io[name] = nc.dram_tensor(name, [cfg.D], F32,
                                      kind="ExternalInput").ap()
    build_layer(nc, cfg, io)
    nc.finalize()  # runs Bacc legalization (wait splitting, regalloc)
    return nc


_CACHE = {}


def kernel(**inputs):
    x = np.asarray(inputs["x"], dtype=np.float32)
    mask = np.asarray(inputs["mask"], dtype=np.float32)
    B, S, D = x.shape
    NQ = mask.shape[1]
    NP = int(np.prod(mask.shape[2:]))
    MLP = inputs["fc1_w"].shape[1]

    cfg = Cfg(
        B=B, S=S, D=D, NP=NP, NQ=NQ, MLP=MLP,
        use_ln1_g=not np.all(np.asarray(inputs["ln1_g"]) == 1.0),
        use_ln1_b=not np.all(np.asarray(inputs["ln1_b"]) == 0.0),
        use_ln2_g=not np.all(np.asarray(inputs["ln2_g"]) == 1.0),
        use_ln2_b=not np.all(np.asarray(inputs["ln2_b"]) == 0.0),
        use_proj_b=not np.all(np.asarray(inputs["proj_b"]) == 0.0),
        use_fc2_b=not np.all(np.asarray(inputs["fc2_b"]) == 0.0),
    )
    key = cfg.key()
    if key not in _CACHE:
        _CACHE[key] = build_full(cfg)
    nc = _CACHE[key]

    bf = ml_dtypes.bfloat16
    shared = {
        "qkv_w": np.ascontiguousarray(np.asarray(inputs["qkv_w"]).astype(bf)),
        "proj_w": np.ascontiguousarray(np.asarray(inputs["proj_w"]).astype(bf)),
        "fc1_w": np.ascontiguousarray(
            (np.asarray(inputs["fc1_w"], dtype=np.float32) * 64.0).astype(
                ml_dtypes.float8_e4m3fn)).view(np.uint8),
        "fc2_w": np.ascontiguousarray(np.asarray(inputs["fc2_w"]).astype(bf)),
        "fc1_b": np.ascontiguousarray(np.asarray(inputs["fc1_b"],
                                                 dtype=np.float32)),
    }
    for flag, name in [(cfg.use_ln1_g, "ln1_g"), (cfg.use_ln1_b, "ln1_b"),
                       (cfg.use_ln2_g, "ln2_g"), (cfg.use_ln2_b, "ln2_b"),
                       (cfg.use_proj_b, "proj_b"), (cfg.use_fc2_b, "fc2_b")]:
        if flag:
            shared[name] = np.ascontiguousarray(
                np.asarray(inputs[name], dtype=np.float32))

    in_maps = []
    for b in range(B):
        m = dict(shared)
        m["x"] = np.ascontiguousarray(x[b])
        m["maskT"] = np.ascontiguousarray(
            mask[b].reshape(NQ, NP).T.astype(np.float32))
        in_maps.append(m)

    from concourse.bass_utils import run_bass_kernel_spmd
    res = run_bass_kernel_spmd(nc, in_maps, core_ids=list(range(B)))
    return np.stack([res.results[b]["out"] for b in range(B)], axis=0)


if __name__ == "__main__":
    cfg = Cfg()
    nc = build_full(cfg)
    print("built ok")
